# revision 18
# baseline (speedup 1.0000x reference)
"""Causal self-attention (B=4, T=2048, C=768, H=12) on 8 trn2 NeuronCores.

Sharding: core c -> batch c//2, head-group c%2 (6 heads each).
Each core computes qkv projection, flash-style causal attention and its
partial c_proj contribution for its 6 heads; the host sums the two
head-group partials per batch and adds b_proj.

Schedule notes (v5):
- EXP batching: S lives in 2-bank PSUM tiles [128, 1024] covering two
  128-key blocks; ONE Exp ACTIVATE covers the pair (two for diagonal
  pairs, split at the bank edge).  The scalar engine's ~352-cycle
  per-ACTIVATE overhead was >half its total time with per-block exps.
- The causal triangle mask is applied INSIDE the S accumulation with an
  extra matmul (stationary -1e4*I, moving strict-lower-tri ones), so
  exp() of masked entries is exactly 0 and gpsimd leaves the
  S->exp->attV critical chain.
- Q tiles hold BOTH heads of a pair (K tiles keep the zeroed half), so
  S still contracts over the full 128 partitions (full PE clock) with
  half the q bias-copies.
- Softmax normalize: DVE copy + reciprocal_approx_fast + gpsimd
  partition_broadcast + DVE multiply -- no DMA in the loop.
- qi-MAJOR schedule (all 6 heads per query-chunk stripe): spreads the
  scalar(exp) load and the filler demand evenly; c_proj for stripe
  qi-1 becomes PE filler during stripe qi.
- Fillers (v blocks, later qkv chains, proj) are injected between
  S-groups at a measured rate (remaining fillers / remaining groups)
  so the PE never starves late in the run.
- Host packs all inputs in the exact SBUF layout, so every load DMA is
  a contiguous-line slice; output stores are bf16, 1 DMA / 128 tokens.
- attV lags TWO S-groups (pp pool buffers p tiles) to absorb
  exp-latency jitter; HAM warm-up dummy matmuls keep the PE clock gate
  open from the first real matmul; boot emission order matches the DMA
  ring order (strict FIFO queues); the final proj drain double-buffers
  its accumulators across the then-idle ps_s banks.
"""

import numpy as np
import ml_dtypes

_BF16 = ml_dtypes.bfloat16

B, T, C = 4, 2048, 768
H, HD = 12, 64
NCORES = 8
NH = 6            # heads per core
CQ = NH * HD      # 384
CPAD = 768        # contraction dim
TQ = 512          # query chunk
KCB = 128         # key block
VSTR = NH * (HD + 1)  # 390: v_sb stride per key block (65 per head)
NQC = T // TQ      # 4 query chunks
NCC = CPAD // 128  # 6 contraction chunks
NKC = T // KCB     # 16 key blocks
NEG = -10000.0     # masked-logit additive (exp -> exactly 0)

XTW = NCC * TQ          # 3072: xt cols per t4 block
WQW = NCC * 256         # 1536: wqk cols per pair

_cache = {}


def _build():
    import concourse.bacc as bacc
    import concourse.bass as bass
    from concourse import mybir
    from concourse.tile import TileContext

    f32 = mybir.dt.float32
    bf16 = mybir.dt.bfloat16
    EXP = mybir.ActivationFunctionType.Exp

    nc = bacc.Bacc("TRN2", target_bir_lowering=False, debug=False)
    # all inputs host-packed in SBUF layout (contiguous DMA lines)
    d_xt = nc.dram_tensor("xt", [128, NQC * XTW], bf16, kind="ExternalInput")
    d_wqk = nc.dram_tensor("wqk", [128, 3 * WQW], bf16, kind="ExternalInput")
    d_wv = nc.dram_tensor("wv", [128, NCC * CQ], bf16, kind="ExternalInput")
    d_wp = nc.dram_tensor("wp", [128, 3 * C], bf16, kind="ExternalInput")
    d_cst = nc.dram_tensor("cst", [KCB, 2 * KCB], bf16, kind="ExternalInput")
    d_bqk = nc.dram_tensor("bqk", [128, 11], f32, kind="ExternalInput")
    d_bv = nc.dram_tensor("bv", [1, CQ], f32, kind="ExternalInput")
    d_out = nc.dram_tensor("out", [T, C], bf16, kind="ExternalOutput")

    def xt_col(t4, cc):
        return t4 * XTW + cc * TQ

    def wqk_col(pr, cc, m):
        return pr * WQW + cc * 256 + m * 128

    with TileContext(nc) as tc:
        with tc.tile_pool(name="const", bufs=1) as const:
            xt_sb = const.tile([128, NQC * XTW], bf16, name="xt", tag="xt")
            wqk_sb = const.tile([128, 3 * WQW], bf16, name="wqk", tag="wqk")
            wv_sb = const.tile([128, NCC * CQ], bf16, name="wv", tag="wv")
            wp_sb = const.tile([128, 3 * C], bf16, name="wp", tag="wp")
            cst_sb = const.tile([128, 2 * KCB], bf16, name="cst", tag="cst")
            bqk_sb = const.tile([128, 11], f32, name="bqk", tag="bqk")
            bvb_sb = const.tile([128, CQ], f32, name="bvb", tag="bvb")
            # q tiles: per (pair, chunk), BOTH heads live
            qp_t = [[const.tile([128, TQ], bf16, name=f"q{j}_{t4}", tag=f"q{j}_{t4}")
                     for t4 in range(NQC)] for j in range(3)]
            # k tiles: per (head, chunk), head in its 64-row half, zeros other
            k_t = [[const.tile([128, TQ], bf16, name=f"k{h}_{t4}", tag=f"k{h}_{t4}")
                    for t4 in range(NQC)] for h in range(NH)]
            v_t = [const.tile([128, VSTR], bf16, name=f"v{kc}", tag=f"v{kc}")
                   for kc in range(NKC)]
            yn_sb = [const.tile([128, T], bf16, name=f"yn{i}", tag=f"yn{i}") for i in range(3)]

            # ---- input DMAs: sync HW ring, FIFO -> first-use order.
            # boot consumes (pair, cc) in pair-major order at t4=0.
            nc.sync.dma_start(out=bqk_sb, in_=d_bqk.ap())
            nc.sync.dma_start(out=xt_sb[:, 0:XTW // 2], in_=d_xt.ap()[:, 0:XTW // 2])
            nc.sync.dma_start(out=wqk_sb[:, 0:WQW], in_=d_wqk.ap()[:, 0:WQW])
            nc.sync.dma_start(out=xt_sb[:, XTW // 2:XTW],
                              in_=d_xt.ap()[:, XTW // 2:XTW])
            nc.sync.dma_start(out=wqk_sb[:, WQW:2 * WQW],
                              in_=d_wqk.ap()[:, WQW:2 * WQW])
            nc.sync.dma_start(out=wqk_sb[:, 2 * WQW:3 * WQW],
                              in_=d_wqk.ap()[:, 2 * WQW:3 * WQW])
            nc.sync.dma_start(out=wv_sb, in_=d_wv.ap())
            nc.sync.dma_start(
                out=bvb_sb,
                in_=bass.AP(tensor=d_bv, offset=0, ap=[[0, 128], [1, CQ]]))
            nc.sync.dma_start(out=cst_sb, in_=d_cst.ap())
            for t4 in range(1, NQC):
                nc.sync.dma_start(out=xt_sb[:, t4 * XTW:(t4 + 1) * XTW],
                                  in_=d_xt.ap()[:, t4 * XTW:(t4 + 1) * XTW])
            # wp is loaded later (only needed by proj).

            # HAM warm-up: dummy matmuls on scratch SBUF while boot DMAs
            # are in flight, so the PE clock gate (1.2->2.4 GHz) is already
            # open when the first real matmul issues.
            scr = const.tile([128, TQ], bf16, name="scr", tag="scr")
            nc.vector.memset(scr, 0.0)
            with tc.tile_pool(name="warm", bufs=1, space="PSUM") as warmp:
                wt = warmp.tile([128, TQ], f32, name="wt", tag="wt")
                for _ in range(16):
                    nc.tensor.matmul(wt[:, 0:TQ], lhsT=scr[:, 0:128],
                                     rhs=scr, start=True, stop=True)

            # ones column per (key block, head)
            for kc in range(NKC):
                v_ones = v_t[kc].rearrange("p (h e) -> p h e", e=HD + 1)[:, :, HD:HD + 1]
                nc.gpsimd.memset(v_ones, 1.0)

            # bias copies: q gets ONE full-pair add; k gets mask+masked-bias
            def q_bias_copy(j, t4, ps, pcols):
                nc.vector.tensor_scalar(
                    out=qp_t[j][t4], in0=ps[:, pcols],
                    scalar1=bqk_sb[:, j:j + 1], scalar2=None,
                    op0=mybir.AluOpType.add)

            def k_bias_copy(j, t4, ps, pcols):
                for half in range(2):
                    nc.vector.tensor_scalar(
                        out=k_t[2 * j + half][t4],
                        in0=ps[:, pcols],
                        scalar1=bqk_sb[:, 9 + half:10 + half],
                        scalar2=bqk_sb[:, 3 + 2 * j + half:4 + 2 * j + half],
                        op0=mybir.AluOpType.mult,
                        op1=mybir.AluOpType.add)

            with tc.tile_pool(name="work", bufs=2, space="PSUM") as work:

                vdone = [False] * NKC

                def emit_vblk(kc):
                    if vdone[kc]:
                        return False
                    vdone[kc] = True
                    t4k, kb = kc // 4, 128 * (kc % 4)
                    psv = work.tile([128, TQ], f32, name="psv", tag="w")
                    for cc in range(NCC):
                        nc.tensor.matmul(
                            psv[:, 0:CQ],
                            lhsT=xt_sb[:, xt_col(t4k, cc) + kb:xt_col(t4k, cc) + kb + 128],
                            rhs=wv_sb[:, CQ * cc:CQ * (cc + 1)],
                            start=(cc == 0), stop=(cc == NCC - 1),
                        )
                    dst = v_t[kc].rearrange("p (h e) -> p h e", e=HD + 1)[:, :, 0:HD]
                    nc.vector.scalar_tensor_tensor(
                        out=dst,
                        in0=psv[:, 0:CQ].rearrange("p (h e) -> p h e", e=HD),
                        scalar=0.0,
                        in1=bvb_sb.rearrange("p (h e) -> p h e", e=HD),
                        op0=mybir.AluOpType.add, op1=mybir.AluOpType.add)
                    return True

                qkvdone = {}

                def emit_qkv_chain(jc, t4):
                    # jc in 0..5: pair j = jc%3, member m = jc//3 (0=q, 1=k)
                    if qkvdone.get((jc, t4)):
                        return False
                    qkvdone[(jc, t4)] = True
                    j, m = jc % 3, jc // 3
                    ps = work.tile([128, TQ], f32, name="qkv", tag="w")
                    for cc in range(NCC):
                        nc.tensor.matmul(
                            ps[:, 0:TQ],
                            lhsT=wqk_sb[:, wqk_col(j, cc, m):wqk_col(j, cc, m) + 128],
                            rhs=xt_sb[:, xt_col(t4, cc):xt_col(t4, cc) + TQ],
                            start=(cc == 0), stop=(cc == NCC - 1),
                        )
                    if m == 0:
                        q_bias_copy(j, t4, ps, slice(0, TQ))
                    else:
                        k_bias_copy(j, t4, ps, slice(0, TQ))
                    return True

                # boot: q+k chains at t4=0 for ALL three pairs (3x 2-bank
                # PSUM tiles + work's 2 banks), pair-major so each pair's
                # chains start as soon as its wqk slice arrives.
                with tc.tile_pool(name="boot", bufs=1, space="PSUM") as bootp:
                    btiles = [bootp.tile([128, 2 * TQ], f32, name=f"bt{j}", tag=f"bt{j}")
                              for j in range(3)]
                    for j in range(3):
                        for cc in range(NCC):
                            for m in range(2):
                                nc.tensor.matmul(
                                    btiles[j][:, TQ * m:TQ * (m + 1)],
                                    lhsT=wqk_sb[:, wqk_col(j, cc, m):wqk_col(j, cc, m) + 128],
                                    rhs=xt_sb[:, xt_col(0, cc):xt_col(0, cc) + TQ],
                                    start=(cc == 0), stop=(cc == NCC - 1),
                                )
                        qkvdone[(j, 0)] = qkvdone[(3 + j, 0)] = True
                        q_bias_copy(j, 0, btiles[j], slice(0, TQ))
                        k_bias_copy(j, 0, btiles[j], slice(TQ, 2 * TQ))
                    # v blocks AFTER all pair chains: wv arrives after wqk in
                    # the DMA ring, and the tensor queue is strict FIFO -- a
                    # v matmul waiting on wv must not sit ahead of pair-1/2
                    # chains whose data is already there.
                    for kc in range(4):
                        emit_vblk(kc)

                # attention pools: 2(work) + 4(ps_s) + 2(ps_y) = 8 PSUM banks
                with tc.tile_pool(name="ps_s", bufs=2, space="PSUM") as ps_s, \
                     tc.tile_pool(name="ps_y", bufs=2, space="PSUM") as ps_y, \
                     tc.tile_pool(name="pp", bufs=4) as pp, \
                     tc.tile_pool(name="smalls", bufs=6) as smalls, \
                     tc.tile_pool(name="outp", bufs=6) as outp:

                    projdone = [False] * (T // 128)
                    pending_stores = []

                    def emit_proj(tcb, alt_pool=None):
                        if projdone[tcb]:
                            return False
                        projdone[tcb] = True
                        ob = outp.tile([128, C], bf16, name="ob", tag="ob")
                        for oc in range(2):
                            if alt_pool is not None and oc == 1:
                                po = alt_pool.tile([128, 2 * TQ], f32,
                                                   name="sg", tag="sg")
                            else:
                                po = work.tile([128, TQ], f32, name="po", tag="w")
                            for fcc in range(3):
                                nc.tensor.matmul(
                                    po[:, 0:CQ],
                                    lhsT=yn_sb[fcc][:, 128 * tcb:128 * (tcb + 1)],
                                    rhs=wp_sb[:, C * fcc + CQ * oc:C * fcc + CQ * (oc + 1)],
                                    start=(fcc == 0), stop=(fcc == 2),
                                )
                            nc.vector.tensor_copy(ob[:, CQ * oc:CQ * (oc + 1)], po[:, 0:CQ])
                        pending_stores.append((tcb, ob))
                        return True

                    def flush_stores(limit=None):
                        n = 0
                        while pending_stores and (limit is None or n < limit):
                            n += 1
                            tcb, ob = pending_stores.pop(0)
                            nc.sync.dma_start(
                                out=d_out.ap()[128 * tcb:128 * (tcb + 1), :], in_=ob)

                    # paced filler injection: consume fillq at
                    # (remaining fillers) / (remaining S-groups)
                    fillq = []
                    groups_left = [sum(2 * qi + 2 for qi in range(NQC)) * NH]
                    credit = [0.0]

                    def fill_tick():
                        if groups_left[0] > 0:
                            credit[0] += len(fillq) / groups_left[0]
                            groups_left[0] -= 1
                        while credit[0] >= 1.0 and fillq:
                            if fillq.pop(0)():
                                credit[0] -= 1.0

                    # ---- one attention unit: head h, query chunk qi ----
                    def emit_attn_unit(h, qi):
                        flush_stores(limit=2)
                        j = h // 2
                        emit_qkv_chain(j, qi)
                        emit_qkv_chain(3 + j, qi)
                        for kc in range(4 * qi + 4):
                            emit_vblk(kc)
                        nkc = 4 * qi + 4
                        y = ps_y.tile([HD + 1, TQ], f32, name="y", tag="y")
                        pend = []  # attV lags one S-pair behind

                        def emit_attv(p, g):
                            for idx in range(2):
                                kc = 2 * g + idx
                                r = kc - 4 * qi
                                off = KCB * r if r >= 0 else 0
                                nc.tensor.matmul(
                                    y[:, off:TQ],
                                    lhsT=v_t[kc][:, (HD + 1) * h:(HD + 1) * (h + 1)],
                                    rhs=p[:, TQ * idx + off:TQ * (idx + 1)],
                                    start=(kc == 0), stop=(kc == nkc - 1),
                                )

                        for g in range(nkc // 2):
                            kcs = (2 * g, 2 * g + 1)
                            rs = tuple(kc - 4 * qi for kc in kcs)
                            sg = ps_s.tile([128, 2 * TQ], f32, name="sg", tag="sg")
                            p = pp.tile([128, 2 * TQ], bf16, name="p", tag="p")
                            for idx, (kc, r) in enumerate(zip(kcs, rs)):
                                off = KCB * r if r >= 0 else 0
                                nc.tensor.matmul(
                                    sg[:, TQ * idx + off:TQ * (idx + 1)],
                                    lhsT=k_t[h][kc // 4][:, KCB * (kc % 4):KCB * (kc % 4 + 1)],
                                    rhs=qp_t[j][qi][:, off:TQ],
                                    start=True, stop=(r < 0),
                                )
                                if r >= 0:  # diagonal: add -1e4 * strict upper tri
                                    nc.tensor.matmul(
                                        sg[:, TQ * idx + off:TQ * idx + off + KCB],
                                        lhsT=cst_sb[:, KCB:2 * KCB],
                                        rhs=cst_sb[:, 0:KCB],
                                        start=False, stop=True,
                                    )
                            if rs[0] >= 0:  # diagonal pair: one ACT spanning
                                # both banks; the [TQ, TQ+o1) strip exps stale
                                # PSUM but nothing reads that p region.
                                o0, o1 = KCB * rs[0], KCB * rs[1]
                                nc.scalar.activation(p[:, o0:2 * TQ],
                                                     sg[:, o0:2 * TQ], EXP)
                            elif False:
                                pass
                            else:
                                nc.scalar.activation(p[:, 0:2 * TQ], sg[:, 0:2 * TQ], EXP)
                            pend.append((p, g))
                            if len(pend) > 2:
                                emit_attv(*pend.pop(0))
                            fill_tick()
                        while pend:
                            emit_attv(*pend.pop(0))
                        # normalize by softmax denominator (row HD of y)
                        rc = smalls.tile([1, TQ], f32, name="rc", tag="rc")
                        nc.vector.tensor_copy(rc, y[HD:HD + 1, :])
                        rinv = smalls.tile([1, TQ], f32, name="rinv", tag="rinv")
                        nc.vector.reciprocal_approx_fast(out=rinv, in_=rc)
                        rb = smalls.tile([HD, TQ], f32, name="rb", tag="rb")
                        nc.gpsimd.partition_broadcast(rb, rinv)
                        fc, half = h // 2, h % 2
                        q0 = TQ * qi
                        nc.vector.tensor_mul(
                            yn_sb[fc][HD * half:HD * (half + 1), q0:q0 + TQ],
                            y[0:HD, :], rb)

                    # filler queue in need-order (stripe 1 first)
                    for t4 in range(1, NQC):
                        for kc in range(4 * t4, 4 * t4 + 4):
                            fillq.append(lambda kc=kc: emit_vblk(kc))
                        for jc in (0, 3, 1, 4, 2, 5):
                            fillq.append(lambda jc=jc, t4=t4: emit_qkv_chain(jc, t4))

                    # qi-major stripes over all 6 heads
                    for qi in range(NQC):
                        for h in range(NH):
                            emit_attn_unit(h, qi)
                        if qi == 0:  # wp load: ring quiet, proj needed stripe 1+
                            nc.sync.dma_start(out=wp_sb, in_=d_wp.ap())
                        if qi > 0:  # release previous stripe's proj as filler
                            for tcb in range(4 * (qi - 1), 4 * qi):
                                fillq.append(lambda tcb=tcb: emit_proj(tcb))
                    # drain remaining proj + stores; the drain projs
                    # also use the (now idle) ps_s banks so CAST latency
                    # doesn't serialize consecutive proj blocks.
                    for tcb in range(T // 128):
                        emit_proj(tcb, alt_pool=ps_s)
                        flush_stores()

    nc.compile()
    return nc


def _prep_core(x, w_attn, b_attn, w_proj, c):
    b, g = c // 2, c % 2
    h0 = NH * g
    q = slice(64 * h0, 64 * h0 + CQ)
    k = slice(C + 64 * h0, C + 64 * h0 + CQ)
    v = slice(2 * C + 64 * h0, 2 * C + 64 * h0 + CQ)

    # xt packed [128, t4*3072 + cc*512 + t]
    xt = np.ascontiguousarray(
        x[b].T.reshape(NCC, 128, NQC, TQ).transpose(1, 2, 0, 3).reshape(128, NQC * XTW)
    ).astype(_BF16)

    wq = (w_attn[:, q] * 0.125).astype(np.float32)   # [768, 384]
    wk = w_attn[:, k].astype(np.float32)
    # pair-ordered columns then packed [128, pr*1536 + cc*256 + m*128 + i]
    wqk2 = np.empty((CPAD, 2 * CQ), dtype=np.float32)
    for j in range(3):
        wqk2[:, 256 * j:256 * j + 128] = wq[:, 128 * j:128 * (j + 1)]
        wqk2[:, 256 * j + 128:256 * (j + 1)] = wk[:, 128 * j:128 * (j + 1)]
    wqk = np.ascontiguousarray(
        wqk2.reshape(NCC, 128, 3, 256).transpose(1, 2, 0, 3).reshape(128, 3 * WQW)
    ).astype(_BF16)

    bq6 = (b_attn[q] * 0.125).astype(np.float32).reshape(3, 128).T   # [128, 3]
    bk6 = b_attn[k].astype(np.float32).reshape(3, 128).T
    m0 = np.zeros((128, 1), np.float32); m0[0:64] = 1.0
    m1 = 1.0 - m0
    bqk = np.empty((128, 11), np.float32)
    bqk[:, 0:3] = bq6                      # full-pair q biases
    for j in range(3):                     # masked k biases (even/odd half)
        bqk[:, 3 + 2 * j:4 + 2 * j] = bk6[:, j:j + 1] * m0
        bqk[:, 4 + 2 * j:5 + 2 * j] = bk6[:, j:j + 1] * m1
    bqk[:, 9:10] = m0                      # half-masks
    bqk[:, 10:11] = m1

    wv = np.ascontiguousarray(
        w_attn[:, v].astype(np.float32).reshape(NCC, 128, CQ)
        .transpose(1, 0, 2).reshape(128, NCC * CQ)).astype(_BF16)
    bv = np.ascontiguousarray(b_attn[v].reshape(1, CQ)).astype(np.float32)

    wp = np.ascontiguousarray(
        w_proj[q, :].astype(np.float32).reshape(3, 128, C)
        .transpose(1, 0, 2).reshape(128, 3 * C)).astype(_BF16)

    ii = np.arange(KCB)
    cst = np.zeros((KCB, 2 * KCB), np.float32)
    cst[:, 0:KCB] = (ii[:, None] > ii[None, :]).astype(np.float32)  # strict tril
    cst[ii, KCB + ii] = NEG                                          # NEG * I
    return {"xt": xt, "wqk": wqk, "wv": wv, "wp": wp,
            "cst": cst.astype(_BF16), "bqk": bqk, "bv": bv}


def kernel(x, w_attn, b_attn, w_proj, b_proj):
    from concourse.bass_utils import run_bass_kernel_spmd

    x = np.asarray(x, dtype=np.float32)
    w_attn = np.asarray(w_attn, dtype=np.float32)
    b_attn = np.asarray(b_attn, dtype=np.float32)
    w_proj = np.asarray(w_proj, dtype=np.float32)
    b_proj = np.asarray(b_proj, dtype=np.float32)

    if "nc" not in _cache:
        _cache["nc"] = _build()
    nc = _cache["nc"]

    in_maps = [_prep_core(x, w_attn, b_attn, w_proj, c) for c in range(NCORES)]
    res = run_bass_kernel_spmd(nc, in_maps, core_ids=list(range(NCORES)))

    out = np.empty((B, T, C), dtype=np.float32)
    for b in range(B):
        out[b] = (res.results[2 * b]["out"].astype(np.float32)
                  + res.results[2 * b + 1]["out"].astype(np.float32) + b_proj)
    return out
